# revision 12
# baseline (speedup 1.0000x reference)
"""Trainium2 Bass kernel for MAGAT (multi-asset GAT portfolio model).

Strategy: pure data-parallel over batch B=32 across 8 NeuronCores (4 samples
each). Per core, all activations are kept feature-major ("T layout": features
on SBUF partitions, tokens on the free dim) so every matmul contraction sits
on the partition dim with zero on-device transposition of the big input (the
host feeds x_asset already feature-major per core).

Attention trick: with x = s_i + t_j,
    exp(leaky_relu(x)) = exp(0.2 s_i) * max(exp(t_j + 0.8 s_i), exp(0.2 t_j))
and the per-column factor exp(0.2 s_i) cancels in softmax, so each 128x512
attention tile is ONE fused op: (z_bcast * q_j) max u'_j, where
z_bcast = ones x exp(0.8 s) is a rank-1 PE matmul, q = exp(t), u' = exp(0.2 t).
Row sums for softmax come free via a ones-column appended to the Wx operand of
the A@Wx matmul. A fraction of tiles instead runs on the Scalar engine as
relu(z*q - u') with a rank-1 PSUM correction, to split the elementwise load.
"""

import numpy as np

import concourse.bass as bass
import concourse.bacc as bacc
import concourse.tile as tile
import concourse.mybir as mybir
import concourse.bass_utils as bass_utils
from concourse.mybir import ActivationFunctionType as AF, AluOpType as ALU

fp32 = mybir.dt.float32

N_CORES = 8
B, NA, L, F = 32, 512, 60, 16
NM, GH, H, MH, PH, V = 32, 64, 4, 32, 128, 512
BL = B // N_CORES          # samples per core = 4
R = BL * NA                # tokens per core = 2048
KD = L * F                 # 960
KC, KP = 8, 120            # mm1 contraction chunks: 8 x 120
NCH = R // 128             # 16 token chunks
D1, D2 = GH // H, GH       # head dims: 16, 64

# fraction of attention units (out of 16 per layer) on the ScalarE path
ACT_OF_16 = 12


def _act(nc, out, in_, func, bias=0.0, scale=1.0):
    nc.scalar.activation(out, in_, func, bias=bias, scale=scale)


def _ln_T(nc, pools, x_sb, out_sb, nfeat, ncols, inv_sb, eps_sb, g_sb, b_sb, pbase=0):
    """LayerNorm over the partition (feature) dim for T-layout tiles.

    x_sb/out_sb: (nfeat, ncols) at partition base pbase. inv_sb: (nfeat, nfeat)
    matmul lhsT filled with 1/nfeat (at base 0). Stats via PE (replicated
    mean), then DVE/ACT elementwise.
    """
    scr, ps_a, ps_b = pools["scr"], pools["ps_tn"], pools["ps_small"]
    sl = slice(pbase, pbase + nfeat)
    mean_ps = ps_a.tile([pbase + nfeat, ncols], fp32, tag="tn")
    nc.tensor.matmul(mean_ps[sl, :], inv_sb, x_sb, start=True, stop=True)
    sq = scr.tile([pbase + nfeat, ncols], fp32, tag=f"lnsq{pbase}")
    _act(nc, sq[sl, :], x_sb, AF.Square)
    msq_ps = ps_b.tile([pbase + nfeat, ncols], fp32, tag="small")
    nc.tensor.matmul(msq_ps[sl, :], inv_sb, sq[sl, :], start=True, stop=True)
    m2 = scr.tile([pbase + nfeat, ncols], fp32, tag=f"lnm2{pbase}")
    _act(nc, m2[sl, :], mean_ps[sl, :], AF.Square)
    var = scr.tile([pbase + nfeat, ncols], fp32, tag=f"lnvar{pbase}")
    nc.vector.tensor_sub(var[sl, :], msq_ps[sl, :], m2[sl, :])
    _act(nc, var[sl, :], var[sl, :], AF.Sqrt, bias=eps_sb)
    rstd = scr.tile([pbase + nfeat, ncols], fp32, tag=f"lnrs{pbase}")
    nc.vector.reciprocal(rstd[sl, :], var[sl, :])
    xc = scr.tile([pbase + nfeat, ncols], fp32, tag=f"lnxc{pbase}")
    nc.vector.tensor_sub(xc[sl, :], x_sb, mean_ps[sl, :])
    nc.vector.tensor_mul(xc[sl, :], xc[sl, :], rstd[sl, :])
    nc.vector.tensor_scalar(out_sb, xc[sl, :], g_sb, b_sb, ALU.mult, ALU.add)


def build_program(debug=False, gelu_fn=AF.Gelu):
    nc = bacc.Bacc("TRN2", target_bir_lowering=False, debug=False,
                   enable_asserts=True, num_devices=N_CORES)

    ins = {}
    def din(name, shape):
        ins[name] = nc.dram_tensor(name, shape, fp32, kind="ExternalInput").ap()
        return ins[name]

    xaT_d = din("xaT", [KD, R])
    xmT_d = din("xmT", [NM, BL * L])
    w1_d = din("ae_w1", [KD, 2 * GH])
    b1_d = din("ae_b1c", [2 * GH, 1])
    w2_d = din("ae_w2", [2 * GH, GH])
    b2_d = din("ae_b2c", [GH, 1])
    aeg_d = din("ae_gc", [GH, 1])
    aebt_d = din("ae_btc", [GH, 1])
    mew1_d = din("me_w1", [NM, MH])
    meb1_d = din("me_b1c", [MH, 1])
    mew2_d = din("me_w2", [MH, MH])
    meb2_d = din("me_b2c", [MH, 1])
    meg_d = din("me_gc", [MH, 1])
    mebt_d = din("me_btc", [MH, 1])
    g1w_d = din("g1_w", [GH, GH])
    g2w_d = din("g2_w", [GH, H * GH])
    wt1_d = din("wt1", [GH, 8])        # cols 0-3 s-heads, 4-7 t-heads
    wt2_d = din("wt2", [GH, 8])
    wts1_d = din("wts1", [GH, 97])     # col 32h = s-head h (spread), else 0
    wts2_d = din("wts2", [GH, 97])
    gng_d = din("gn_g_row", [1, GH])
    gnb_d = din("gn_b_row", [1, GH])
    phw1_d = din("ph_w1", [GH + MH, PH])
    phb1_d = din("ph_b1c", [PH, 1])
    phw2_d = din("ph_w2", [PH, PH // 2])
    phb2_d = din("ph_b2c", [PH // 2, 1])
    phw3_d = din("ph_w3", [PH // 2, V])
    phb3_d = din("ph_b3_row", [1, V])
    ident_d = din("ident", [128, 128])

    out_d = nc.dram_tensor("out", [BL, V], fp32, kind="ExternalOutput").ap()
    dbg = {}
    if debug:
        for nm, shp in [("d_embT", [GH, R]), ("d_h1gT", [GH, R]),
                        ("d_ge", [NCH, 128, GH]), ("d_mac", [MH, BL])]:
            dbg[nm] = nc.dram_tensor(nm, shp, fp32, kind="ExternalOutput").ap()

    with tile.TileContext(nc) as tc:
        _emit(tc, ins, out_d, dbg, gelu_fn)
    nc.compile()
    return nc


def _emit(tc, ins, out_d, dbg, gelu_fn=AF.Gelu):
    nc = tc.nc
    import contextlib
    es = contextlib.ExitStack()
    with es:
        consts = es.enter_context(tc.tile_pool(name="consts", bufs=1))
        persist = es.enter_context(tc.tile_pool(name="persist", bufs=1))
        scr = es.enter_context(tc.tile_pool(name="scr", bufs=3))
        ps_big = es.enter_context(tc.tile_pool(name="ps_big", bufs=2, space="PSUM"))
        ps_out = es.enter_context(tc.tile_pool(name="ps_out", bufs=2, space="PSUM"))
        ps_tn = es.enter_context(tc.tile_pool(name="ps_tn", bufs=2, space="PSUM"))
        ps_small = es.enter_context(tc.tile_pool(name="ps_small", bufs=2, space="PSUM"))
        pools = {"scr": scr, "ps_big": ps_big, "ps_out": ps_out,
                 "ps_tn": ps_tn, "ps_small": ps_small}

        # ---------------- constants ----------------
        def load(name, shape, pslice=None):
            t = consts.tile(shape, fp32, tag=name)
            dst = t if pslice is None else t[pslice, :]
            nc.sync.dma_start(out=dst, in_=ins[name])
            return t

        w1_sb = consts.tile([KP, KC, 2 * GH], fp32, tag="w1")
        nc.sync.dma_start(out=w1_sb, in_=ins["ae_w1"].rearrange("(c p) n -> p c n", p=KP))
        w2_sb = load("ae_w2", [2 * GH, GH])
        b1c = load("ae_b1c", [2 * GH, 1])
        b2c = load("ae_b2c", [GH, 1])
        aeg = load("ae_gc", [GH, 1])
        aebt = load("ae_btc", [GH, 1])
        g1w_sb = load("g1_w", [GH, GH])
        g2w_sb = load("g2_w", [GH, H * GH])
        wt_sb = {1: load("wt1", [GH, 8]), 2: load("wt2", [GH, 8])}
        wts_sb = {1: load("wts1", [GH, 97]), 2: load("wts2", [GH, 97])}
        phw1_sb = load("ph_w1", [GH + MH, PH])
        phb1c = load("ph_b1c", [PH, 1])
        phw2_sb = load("ph_w2", [PH, PH // 2])
        phb2c = load("ph_b2c", [PH // 2, 1])
        phw3_sb = load("ph_w3", [PH // 2, V])
        ident = load("ident", [128, 128])
        # macro constants live at partition base 64 (96-partition tiles)
        mew1_sb = consts.tile([96, MH], fp32, tag="me_w1")
        nc.sync.dma_start(out=mew1_sb[64:96, :], in_=ins["me_w1"])
        mew2_sb = consts.tile([96, MH], fp32, tag="me_w2")
        nc.sync.dma_start(out=mew2_sb[64:96, :], in_=ins["me_w2"])
        meb1c = consts.tile([96, 1], fp32, tag="me_b1c")
        nc.sync.dma_start(out=meb1c[64:96, :], in_=ins["me_b1c"])
        meb2c = consts.tile([96, 1], fp32, tag="me_b2c")
        nc.sync.dma_start(out=meb2c[64:96, :], in_=ins["me_b2c"])
        megc = consts.tile([96, 1], fp32, tag="me_gc")
        nc.sync.dma_start(out=megc[64:96, :], in_=ins["me_gc"])
        mebtc = consts.tile([96, 1], fp32, tag="me_btc")
        nc.sync.dma_start(out=mebtc[64:96, :], in_=ins["me_btc"])
        # broadcasts
        gng_b = consts.tile([128, GH], fp32, tag="gng_b")
        nc.gpsimd.dma_start(out=gng_b, in_=ins["gn_g_row"].to_broadcast((128, GH)))
        gnb_b = consts.tile([128, GH], fp32, tag="gnb_b")
        nc.gpsimd.dma_start(out=gnb_b, in_=ins["gn_b_row"].to_broadcast((128, GH)))
        b3_b = consts.tile([BL, V], fp32, tag="b3_b")
        nc.gpsimd.dma_start(out=b3_b, in_=ins["ph_b3_row"].to_broadcast((BL, V)))
        # small synthesized constants
        inv64 = consts.tile([GH, GH], fp32, tag="inv64")
        nc.vector.memset(inv64, 1.0 / GH)
        inv32 = consts.tile([96, MH], fp32, tag="inv32")
        nc.vector.memset(inv32[64:96, :], 1.0 / MH)
        eps64 = consts.tile([GH, 1], fp32, tag="eps64")
        nc.vector.memset(eps64, 1e-5)
        eps96 = consts.tile([96, 1], fp32, tag="eps96")
        nc.vector.memset(eps96, 1e-5)
        eps128 = consts.tile([128, 1], fp32, tag="eps128")
        nc.vector.memset(eps128, 1e-5)
        ones_r = consts.tile([97, 128], fp32, tag="ones_r")
        nc.vector.memset(ones_r, 1.0)
        ones_c = consts.tile([128, 1], fp32, tag="ones_c")
        nc.vector.memset(ones_c, 1.0)

        # ---------------- asset encoder ----------------
        xa_es = contextlib.ExitStack()
        xa_pool = xa_es.enter_context(tc.tile_pool(name="xa", bufs=1))
        enc_pool = xa_es.enter_context(tc.tile_pool(name="enc", bufs=1))
        xaT_sb = xa_pool.tile([KP, KC, R], fp32)
        for kc in range(KC):
            nc.sync.dma_start(
                out=xaT_sb[:, kc, :],
                in_=ins["xaT"][kc * KP:(kc + 1) * KP, :])
        h1T = enc_pool.tile([2 * GH, R], fp32)
        embpre = enc_pool.tile([GH, R], fp32)
        embT = persist.tile([GH, R], fp32)
        for pass_ in range(2):
            h1ps = []
            for pi in range(2):
                h1ps.append(ps_big.tile([128, 512], fp32, tag="zb", name="h1ps"))
            for kc in range(KC):
                for pi in range(2):
                    p = pass_ * 2 + pi
                    nc.tensor.matmul(h1ps[pi], w1_sb[:, kc, :],
                                     xaT_sb[:, kc, p * 512:(p + 1) * 512],
                                     start=(kc == 0), stop=(kc == KC - 1))
            for pi in range(2):
                p = pass_ * 2 + pi
                _act(nc, h1T[:, p * 512:(p + 1) * 512], h1ps[pi], gelu_fn, bias=b1c)
        for p in range(4):
            sl = slice(p * 512, (p + 1) * 512)
            m2ps = ps_out.tile([GH, 512], fp32, tag="outT")
            nc.tensor.matmul(m2ps, w2_sb, h1T[:, sl], start=True, stop=True)
            _act(nc, embpre[:, sl], m2ps, gelu_fn, bias=b2c)
            _ln_T(nc, pools, embpre[:, sl], embT[:, sl], GH, 512,
                  inv64, eps64, aeg, aebt)
        if dbg:
            nc.sync.dma_start(out=dbg["d_embT"], in_=embT)
        xa_es.close()

        # ---------------- macro encoder (partitions 64:96) ----------------
        mac_pool = tc.tile_pool(name="mac", bufs=1)
        with mac_pool as macp:
            xmT_sb = macp.tile([96, BL * L], fp32)
            nc.sync.dma_start(out=xmT_sb[64:96, :], in_=ins["xmT"])
            m1ps = ps_small.tile([96, BL * L], fp32, tag="small")
            nc.tensor.matmul(m1ps[64:96, :], mew1_sb[64:96, :], xmT_sb[64:96, :],
                             start=True, stop=True)
            m1s = macp.tile([96, BL * L], fp32)
            _act(nc, m1s[64:96, :], m1ps[64:96, :], gelu_fn, bias=meb1c[64:96, :])
            m2ps = ps_small.tile([96, BL * L], fp32, tag="small")
            nc.tensor.matmul(m2ps[64:96, :], mew2_sb[64:96, :], m1s[64:96, :],
                             start=True, stop=True)
            m2s = macp.tile([96, BL * L], fp32)
            _act(nc, m2s[64:96, :], m2ps[64:96, :], AF.Identity, bias=meb2c[64:96, :])
            macpre = macp.tile([96, BL], fp32)
            nc.vector.tensor_reduce(
                macpre[64:96, :],
                m2s[64:96, :].rearrange("p (b l) -> p b l", b=BL),
                axis=mybir.AxisListType.X, op=ALU.add)
            nc.vector.tensor_scalar(macpre[64:96, :], macpre[64:96, :],
                                    1.0 / L, None, ALU.mult)
            macT = persist.tile([96, BL], fp32)
            _ln_T(nc, pools, macpre[64:96, :], macT[64:96, :], MH, BL,
                  inv32[64:96, :], eps96[64:96, :], megc[64:96, :], mebtc[64:96, :],
                  pbase=64)
            if dbg:
                nc.sync.dma_start(out=dbg["d_mac"], in_=macT[64:96, :])

        # ---------------- GAT layers ----------------
        gat_es = contextlib.ExitStack()
        wxo_pool = gat_es.enter_context(tc.tile_pool(name="wxo", bufs=1))
        qt_pool = gat_es.enter_context(tc.tile_pool(name="qt", bufs=6))
        un_pool = gat_es.enter_context(tc.tile_pool(name="un", bufs=4))
        gp_pool = gat_es.enter_context(tc.tile_pool(name="gatp", bufs=1))

        hg1 = gp_pool.tile([128, NCH, GH], fp32, tag="hg1")
        h1gT = persist.tile([GH, R], fp32)
        h2acc = gp_pool.tile([128, NCH, GH], fp32, tag="h2acc")

        def gat_layer(layer, src_T, D):
            dp1 = D + 1
            gw = g1w_sb if layer == 1 else g2w_sb
            # --- prep: Wx in token-major layout (+ones col), q/u/z from st ---
            wxo = [wxo_pool.tile([128, NCH, dp1], fp32, tag=f"wxo{h}", name=f"wxo{h}")
                   for h in range(H)]
            for h in range(H):
                nc.vector.memset(wxo[h][:, :, D:dp1], 1.0)
            q_sb = gp_pool.tile([128, NCH, H], fp32, tag="q_sb")
            u_sb = gp_pool.tile([128, NCH, H], fp32, tag="u_sb")
            nu_sb = gp_pool.tile([128, NCH, H], fp32, tag="nu_sb")
            z_sb = gp_pool.tile([97, R], fp32, tag="z_sb")
            for rc in range(NCH):
                wxps = ps_out.tile([128, H * D], fp32, tag="outT")
                nc.tensor.matmul(wxps, src_T[:, rc * 128:(rc + 1) * 128], gw,
                                 start=True, stop=True)
                for h in range(H):
                    nc.any.tensor_copy(wxo[h][:, rc, 0:D],
                                       wxps[:, h * D:(h + 1) * D])
                stps = ps_tn.tile([128, 8], fp32, tag="tn")
                nc.tensor.matmul(stps, src_T[:, rc * 128:(rc + 1) * 128],
                                 wt_sb[layer], start=True, stop=True)
                _act(nc, q_sb[:, rc, :], stps[:, 4:8], AF.Exp)
                _act(nc, u_sb[:, rc, :], stps[:, 4:8], AF.Exp, scale=0.2)
            nc.vector.tensor_scalar(nu_sb, u_sb, -1.0, None, ALU.mult)
            for b in range(BL):
                zps = ps_big.tile([97, 512], fp32, tag="zb")
                nc.tensor.matmul(zps, wts_sb[layer],
                                 src_T[:, b * 512:(b + 1) * 512],
                                 start=True, stop=True)
                for h in range(H):
                    _act(nc, z_sb[32 * h:32 * h + 1, b * 512:(b + 1) * 512],
                         zps[32 * h:32 * h + 1, :], AF.Exp, scale=0.8)
            # --- attention units ---
            for b in range(BL):
                for h in range(H):
                    u_idx = b * H + h
                    use_act = (u_idx % 16) < ACT_OF_16
                    zbps = ps_big.tile([128, 512], fp32, tag="zb")
                    nc.tensor.matmul(zbps, ones_r[32 * h:32 * h + 1, :],
                                     z_sb[32 * h:32 * h + 1, b * 512:(b + 1) * 512],
                                     start=True, stop=True,
                                     tile_position=(32 * h, 0))
                    outps = ps_out.tile([dp1, 512], fp32, tag="outT")
                    corr = None
                    if use_act:
                        corrps = ps_small.tile([dp1, 1], fp32, tag="small")
                    for jc in range(BL):
                        rc = b * 4 + jc
                        qt = qt_pool.tile([128, 512], fp32, tag="qt")
                        if use_act:
                            _act(nc, qt, zbps, AF.Relu,
                                 bias=nu_sb[:, rc, h:h + 1],
                                 scale=q_sb[:, rc, h:h + 1])
                        else:
                            nc.vector.tensor_scalar(
                                qt, zbps, q_sb[:, rc, h:h + 1],
                                u_sb[:, rc, h:h + 1], ALU.mult, ALU.max)
                        nc.tensor.matmul(outps, wxo[h][:, rc, :], qt,
                                         start=(jc == 0), stop=(jc == 3))
                        if use_act:
                            nc.tensor.matmul(corrps, wxo[h][:, rc, :],
                                             u_sb[:, rc, h:h + 1],
                                             start=(jc == 0), stop=(jc == 3))
                    outsb = un_pool.tile([dp1, 512], fp32, tag="outsb")
                    if use_act:
                        corr = un_pool.tile([dp1, 1], fp32, tag="corr")
                        nc.vector.tensor_copy(corr, corrps)
                        nc.vector.tensor_scalar(outsb, outps, corr, None, ALU.add)
                    else:
                        _act(nc, outsb, outps, AF.Copy)
                    for ic in range(BL):
                        icg = b * 4 + ic
                        tnps = ps_tn.tile([128, dp1], fp32, tag="tn")
                        nc.tensor.transpose(tnps, outsb[:, ic * 128:(ic + 1) * 128],
                                            ident[:dp1, :dp1])
                        rz = un_pool.tile([128, 1], fp32, tag="rz")
                        nc.vector.reciprocal(rz, tnps[:, D:dp1])
                        if layer == 1:
                            nc.vector.tensor_scalar(
                                hg1[:, icg, h * D:(h + 1) * D], tnps[:, 0:D],
                                rz, None, ALU.mult)
                        else:
                            if h == 0:
                                nc.vector.tensor_scalar(
                                    h2acc[:, icg, :], tnps[:, 0:D],
                                    rz, 0.25, ALU.mult, ALU.mult)
                            else:
                                tmp = un_pool.tile([128, D], fp32, tag="tmp")
                                nc.vector.tensor_scalar(
                                    tmp, tnps[:, 0:D], rz, 0.25, ALU.mult, ALU.mult)
                                nc.vector.tensor_tensor(
                                    h2acc[:, icg, :], h2acc[:, icg, :], tmp, ALU.add)

        def elu_inplace(x_sb):
            ex = gp_pool.tile([128, NCH, GH], fp32, tag="elu_ex")
            _act(nc, ex, x_sb, AF.Exp)
            nc.vector.tensor_scalar(ex, ex, 1.0, None, ALU.subtract)
            rel = gp_pool.tile([128, NCH, GH], fp32, tag="elu_rel")
            nc.vector.tensor_scalar(rel, x_sb, 0.0, None, ALU.max)
            nc.vector.tensor_tensor(x_sb, rel, ex, ALU.min)

        gat_layer(1, embT, D1)
        elu_inplace(hg1)
        for rc in range(NCH):
            tpps = ps_tn.tile([GH, 128], fp32, tag="tn")
            nc.tensor.transpose(tpps, hg1[:, rc, :], ident)
            nc.any.tensor_copy(h1gT[:, rc * 128:(rc + 1) * 128], tpps)
        if dbg:
            nc.sync.dma_start(out=dbg["d_h1gT"], in_=h1gT)

        gat_layer(2, h1gT, D2)
        elu_inplace(h2acc)

        # graph LayerNorm (free-dim LN, natural layout)
        ge = gp_pool.tile([128, NCH, GH], fp32, tag="ge")
        for rc in range(NCH):
            bst = scr.tile([128, 6], fp32, tag="bst")
            nc.vector.bn_stats(bst, h2acc[:, rc, :])
            mv = scr.tile([128, 2], fp32, tag="mv")
            nc.vector.bn_aggr(mv, bst)
            std = scr.tile([128, 1], fp32, tag="std")
            _act(nc, std, mv[:, 1:2], AF.Sqrt, bias=eps128)
            rstd = scr.tile([128, 1], fp32, tag="rstd")
            nc.vector.reciprocal(rstd, std)
            t1 = scr.tile([128, GH], fp32, tag="lngt1")
            nc.vector.tensor_scalar(t1, h2acc[:, rc, :], mv[:, 0:1], rstd,
                                    ALU.subtract, ALU.mult)
            nc.vector.tensor_tensor(t1, t1, gng_b, ALU.mult)
            nc.vector.tensor_tensor(ge[:, rc, :], t1, gnb_b, ALU.add)
            if dbg:
                nc.sync.dma_start(out=dbg["d_ge"][rc, :, :], in_=ge[:, rc, :])
        gat_es.close()

        # ---------------- pool + head ----------------
        with tc.tile_pool(name="head", bufs=1) as hp:
            poolps = ps_out.tile([GH, BL], fp32, tag="outT")
            for b in range(BL):
                for rc4 in range(4):
                    nc.tensor.matmul(poolps[:, b:b + 1], ge[:, b * 4 + rc4, :],
                                     ones_c, start=(rc4 == 0), stop=(rc4 == 3))
            comb = hp.tile([GH + MH, BL], fp32)
            nc.vector.tensor_scalar(comb[0:GH, :], poolps, 1.0 / NA, None, ALU.mult)
            nc.vector.tensor_copy(comb[GH:GH + MH, :], macT[64:96, :])
            z1ps = ps_big.tile([PH, BL], fp32, tag="zb")
            nc.tensor.matmul(z1ps, phw1_sb, comb, start=True, stop=True)
            z1s = hp.tile([PH, BL], fp32)
            _act(nc, z1s, z1ps, gelu_fn, bias=phb1c)
            z2ps = ps_tn.tile([PH // 2, BL], fp32, tag="tn")
            nc.tensor.matmul(z2ps, phw2_sb, z1s, start=True, stop=True)
            z2s = hp.tile([PH // 2, BL], fp32)
            _act(nc, z2s, z2ps, gelu_fn, bias=phb2c)
            lgps = ps_out.tile([BL, V], fp32, tag="outT")
            nc.tensor.matmul(lgps, z2s, phw3_sb, start=True, stop=True)
            sm = hp.tile([BL, V], fp32)
            nc.vector.tensor_add(sm, lgps, b3_b)
            mx = hp.tile([BL, 1], fp32)
            nc.vector.tensor_reduce(mx, sm, axis=mybir.AxisListType.X, op=ALU.max)
            nc.vector.tensor_scalar(mx, mx, -1.0, None, ALU.mult)
            ex = hp.tile([BL, V], fp32)
            _act(nc, ex, sm, AF.Exp, bias=mx)
            sme = hp.tile([BL, 1], fp32)
            nc.vector.tensor_reduce(sme, ex, axis=mybir.AxisListType.X, op=ALU.add)
            rs = hp.tile([BL, 1], fp32)
            nc.vector.reciprocal(rs, sme)
            res = hp.tile([BL, V], fp32)
            nc.vector.tensor_scalar(res, ex, rs, None, ALU.mult)
            nc.sync.dma_start(out=out_d, in_=res)


# ---------------- host side ----------------

def host_prep(inputs):
    """Build the per-core in_maps from full (unsharded) numpy inputs."""
    f32 = np.float32
    x_asset = np.ascontiguousarray(inputs["x_asset"], dtype=f32)
    x_macro = np.ascontiguousarray(inputs["x_macro"], dtype=f32)
    g1_w, g1_a = np.asarray(inputs["g1_w"], f32), np.asarray(inputs["g1_a"], f32)
    g2_w, g2_a = np.asarray(inputs["g2_w"], f32), np.asarray(inputs["g2_a"], f32)

    def fold(gw, ga, D):
        ws = np.stack([gw[:, h * D:(h + 1) * D] @ ga[h, :D] for h in range(H)], 1)
        wtt = np.stack([gw[:, h * D:(h + 1) * D] @ ga[h, D:] for h in range(H)], 1)
        wt = np.concatenate([ws, wtt], axis=1).astype(f32)          # (GH, 8)
        wspread = np.zeros((GH, 97), f32)
        for h in range(H):
            wspread[:, 32 * h] = ws[:, h]
        return np.ascontiguousarray(wt), np.ascontiguousarray(wspread)

    wt1, wts1 = fold(g1_w, g1_a, D1)
    wt2, wts2 = fold(g2_w, g2_a, D2)

    col = lambda v: np.ascontiguousarray(np.asarray(v, f32).reshape(-1, 1))
    row = lambda v: np.ascontiguousarray(np.asarray(v, f32).reshape(1, -1))
    shared = {
        "ae_w1": np.ascontiguousarray(inputs["ae_w1"], f32),
        "ae_b1c": col(inputs["ae_b1"]), "ae_w2": np.ascontiguousarray(inputs["ae_w2"], f32),
        "ae_b2c": col(inputs["ae_b2"]), "ae_gc": col(inputs["ae_g"]),
        "ae_btc": col(inputs["ae_bt"]),
        "me_w1": np.ascontiguousarray(inputs["me_w1"], f32), "me_b1c": col(inputs["me_b1"]),
        "me_w2": np.ascontiguousarray(inputs["me_w2"], f32), "me_b2c": col(inputs["me_b2"]),
        "me_gc": col(inputs["me_g"]), "me_btc": col(inputs["me_bt"]),
        "g1_w": g1_w, "g2_w": g2_w, "wt1": wt1, "wt2": wt2,
        "wts1": wts1, "wts2": wts2,
        "gn_g_row": row(inputs["gn_g"]), "gn_b_row": row(inputs["gn_b"]),
        "ph_w1": np.ascontiguousarray(inputs["ph_w1"], f32), "ph_b1c": col(inputs["ph_b1"]),
        "ph_w2": np.ascontiguousarray(inputs["ph_w2"], f32), "ph_b2c": col(inputs["ph_b2"]),
        "ph_w3": np.ascontiguousarray(inputs["ph_w3"], f32), "ph_b3_row": row(inputs["ph_b3"]),
        "ident": np.eye(128, dtype=f32),
    }
    in_maps = []
    for c in range(N_CORES):
        xa = x_asset[c * BL:(c + 1) * BL].reshape(R, KD)
        xm = x_macro[c * BL:(c + 1) * BL].reshape(BL * L, NM)
        m = dict(shared)
        m["xaT"] = np.ascontiguousarray(xa.T)
        m["xmT"] = np.ascontiguousarray(xm.T)
        in_maps.append(m)
    return in_maps


_NC_CACHE = {}


def _get_program(debug=False):
    if debug not in _NC_CACHE:
        _NC_CACHE[debug] = build_program(debug=debug)
    return _NC_CACHE[debug]


def kernel(**inputs) -> np.ndarray:
    nc = _get_program(debug=False)
    in_maps = host_prep(inputs)
    res = bass_utils.run_bass_kernel_spmd(nc, in_maps, core_ids=list(range(N_CORES)))
    return np.concatenate([res.results[c]["out"] for c in range(N_CORES)], axis=0)


# revision 13
# speedup vs baseline: 2.6905x; 2.6905x over previous
"""Trainium2 Bass kernel for MAGAT (multi-asset GAT portfolio model).

Strategy: pure data-parallel over batch B=32 across 8 NeuronCores (4 samples
each). Per core, all activations are kept feature-major ("T layout": features
on SBUF partitions, tokens on the free dim) so every matmul contraction sits
on the partition dim with zero on-device transposition of the big input (the
host feeds x_asset already feature-major per core).

Attention trick: with x = s_i + t_j,
    exp(leaky_relu(x)) = exp(0.2 s_i) * max(exp(t_j + 0.8 s_i), exp(0.2 t_j))
and the per-column factor exp(0.2 s_i) cancels in softmax, so each 128x512
attention tile is ONE fused op: (z_bcast * q_j) max u'_j, where
z_bcast = ones x exp(0.8 s) is a rank-1 PE matmul, q = exp(t), u' = exp(0.2 t).
Row sums for softmax come free via a ones-column appended to the Wx operand of
the A@Wx matmul. A fraction of tiles instead runs on the Scalar engine as
relu(z*q - u') with a rank-1 PSUM correction, to split the elementwise load.
"""

import numpy as np

import concourse.bass as bass
import concourse.bacc as bacc
import concourse.tile as tile
import concourse.mybir as mybir
import concourse.bass_utils as bass_utils
from concourse.mybir import ActivationFunctionType as AF, AluOpType as ALU

fp32 = mybir.dt.float32
bf16 = mybir.dt.bfloat16

N_CORES = 8
B, NA, L, F = 32, 512, 60, 16
NM, GH, H, MH, PH, V = 32, 64, 4, 32, 128, 512
BL = B // N_CORES          # samples per core = 4
R = BL * NA                # tokens per core = 2048
KD = L * F                 # 960
KC, KP = 8, 120            # mm1 contraction chunks: 8 x 120
NCH = R // 128             # 16 token chunks
D1, D2 = GH // H, GH       # head dims: 16, 64

# fraction of attention units (out of 16 per layer) on the ScalarE path
ACT_OF_16 = 12


def _act(nc, out, in_, func, bias=0.0, scale=1.0):
    nc.scalar.activation(out, in_, func, bias=bias, scale=scale)


def _ln_T(nc, pools, x_sb, out_sb, nfeat, ncols, inv_sb, eps_sb, g_sb, b_sb, pbase=0):
    """LayerNorm over the partition (feature) dim for T-layout tiles.

    x_sb/out_sb: (nfeat, ncols) at partition base pbase. inv_sb: (nfeat, nfeat)
    matmul lhsT filled with 1/nfeat (at base 0). Stats via PE (replicated
    mean), then DVE/ACT elementwise.
    """
    scr, ps_a, ps_b = pools["scr"], pools["ps_tn"], pools["ps_small"]
    sl = slice(pbase, pbase + nfeat)
    mean_ps = ps_a.tile([pbase + nfeat, ncols], fp32, tag="tn")
    nc.tensor.matmul(mean_ps[sl, :], inv_sb, x_sb, start=True, stop=True)
    sq = scr.tile([pbase + nfeat, ncols], fp32, tag=f"lnsq{pbase}")
    _act(nc, sq[sl, :], x_sb, AF.Square)
    msq_ps = ps_b.tile([pbase + nfeat, ncols], fp32, tag="small")
    nc.tensor.matmul(msq_ps[sl, :], inv_sb, sq[sl, :], start=True, stop=True)
    m2 = scr.tile([pbase + nfeat, ncols], fp32, tag=f"lnm2{pbase}")
    _act(nc, m2[sl, :], mean_ps[sl, :], AF.Square)
    var = scr.tile([pbase + nfeat, ncols], fp32, tag=f"lnvar{pbase}")
    nc.vector.tensor_sub(var[sl, :], msq_ps[sl, :], m2[sl, :])
    _act(nc, var[sl, :], var[sl, :], AF.Sqrt, bias=eps_sb)
    rstd = scr.tile([pbase + nfeat, ncols], fp32, tag=f"lnrs{pbase}")
    nc.vector.reciprocal(rstd[sl, :], var[sl, :])
    xc = scr.tile([pbase + nfeat, ncols], fp32, tag=f"lnxc{pbase}")
    nc.vector.tensor_sub(xc[sl, :], x_sb, mean_ps[sl, :])
    nc.vector.tensor_mul(xc[sl, :], xc[sl, :], rstd[sl, :])
    nc.vector.tensor_scalar(out_sb, xc[sl, :], g_sb, b_sb, ALU.mult, ALU.add)


def build_program(debug=False, gelu_fn=AF.Gelu):
    nc = bacc.Bacc("TRN2", target_bir_lowering=False, debug=False,
                   enable_asserts=True, num_devices=N_CORES)

    ins = {}
    def din(name, shape, dt=fp32):
        ins[name] = nc.dram_tensor(name, shape, dt, kind="ExternalInput").ap()
        return ins[name]

    xaT_d = din("xaT", [KD, R], bf16)
    xmT_d = din("xmT", [NM, BL * L])
    w1_d = din("ae_w1", [KD, 2 * GH], bf16)
    b1_d = din("ae_b1c", [2 * GH, 1])
    w2_d = din("ae_w2", [2 * GH, GH])
    b2_d = din("ae_b2c", [GH, 1])
    aeg_d = din("ae_gc", [GH, 1])
    aebt_d = din("ae_btc", [GH, 1])
    mew1_d = din("me_w1", [NM, MH])
    meb1_d = din("me_b1c", [MH, 1])
    mew2_d = din("me_w2", [MH, MH])
    meb2_d = din("me_b2c", [MH, 1])
    meg_d = din("me_gc", [MH, 1])
    mebt_d = din("me_btc", [MH, 1])
    g1w_d = din("g1_w", [GH, GH])
    g2w_d = din("g2_w", [GH, H * GH])
    wt1_d = din("wt1", [GH, 8])        # cols 0-3 s-heads, 4-7 t-heads
    wt2_d = din("wt2", [GH, 8])
    wts1_d = din("wts1", [GH, 97])     # col 32h = s-head h (spread), else 0
    wts2_d = din("wts2", [GH, 97])
    gng_d = din("gn_g_row", [1, GH])
    gnb_d = din("gn_b_row", [1, GH])
    phw1_d = din("ph_w1", [GH + MH, PH])
    phb1_d = din("ph_b1c", [PH, 1])
    phw2_d = din("ph_w2", [PH, PH // 2])
    phb2_d = din("ph_b2c", [PH // 2, 1])
    phw3_d = din("ph_w3", [PH // 2, V])
    phb3_d = din("ph_b3_row", [1, V])
    ident_d = din("ident", [128, 128])

    out_d = nc.dram_tensor("out", [BL, V], fp32, kind="ExternalOutput").ap()
    dbg = {}
    if debug:
        for nm, shp in [("d_embT", [GH, R]), ("d_h1gT", [GH, R]),
                        ("d_ge", [NCH, 128, GH]), ("d_mac", [MH, BL])]:
            dbg[nm] = nc.dram_tensor(nm, shp, fp32, kind="ExternalOutput").ap()

    with tile.TileContext(nc) as tc:
        _emit(tc, ins, out_d, dbg, gelu_fn)
    nc.compile()
    return nc


def _emit(tc, ins, out_d, dbg, gelu_fn=AF.Gelu):
    nc = tc.nc
    import contextlib
    es = contextlib.ExitStack()
    with es:
        consts = es.enter_context(tc.tile_pool(name="consts", bufs=1))
        persist = es.enter_context(tc.tile_pool(name="persist", bufs=1))
        scr = es.enter_context(tc.tile_pool(name="scr", bufs=3))
        ps_big = es.enter_context(tc.tile_pool(name="ps_big", bufs=2, space="PSUM"))
        ps_out = es.enter_context(tc.tile_pool(name="ps_out", bufs=2, space="PSUM"))
        ps_tn = es.enter_context(tc.tile_pool(name="ps_tn", bufs=2, space="PSUM"))
        ps_small = es.enter_context(tc.tile_pool(name="ps_small", bufs=2, space="PSUM"))
        pools = {"scr": scr, "ps_big": ps_big, "ps_out": ps_out,
                 "ps_tn": ps_tn, "ps_small": ps_small}

        # ---------------- constants ----------------
        def load(name, shape, pslice=None):
            t = consts.tile(shape, fp32, tag=name)
            dst = t if pslice is None else t[pslice, :]
            nc.sync.dma_start(out=dst, in_=ins[name])
            return t

        w1_sb = consts.tile([KP, KC, 2 * GH], bf16, tag="w1")
        nc.sync.dma_start(out=w1_sb, in_=ins["ae_w1"].rearrange("(c p) n -> p c n", p=KP))
        w2_sb = load("ae_w2", [2 * GH, GH])
        b1c = load("ae_b1c", [2 * GH, 1])
        b2c = load("ae_b2c", [GH, 1])
        aeg = load("ae_gc", [GH, 1])
        aebt = load("ae_btc", [GH, 1])
        g1w_sb = load("g1_w", [GH, GH])
        g2w_sb = load("g2_w", [GH, H * GH])
        wt_sb = {1: load("wt1", [GH, 8]), 2: load("wt2", [GH, 8])}
        wts_sb = {1: load("wts1", [GH, 97]), 2: load("wts2", [GH, 97])}
        phw1_sb = load("ph_w1", [GH + MH, PH])
        phb1c = load("ph_b1c", [PH, 1])
        phw2_sb = load("ph_w2", [PH, PH // 2])
        phb2c = load("ph_b2c", [PH // 2, 1])
        phw3_sb = load("ph_w3", [PH // 2, V])
        ident = load("ident", [128, 128])
        # macro constants live at partition base 64 (96-partition tiles)
        mew1_sb = consts.tile([96, MH], fp32, tag="me_w1")
        nc.sync.dma_start(out=mew1_sb[64:96, :], in_=ins["me_w1"])
        mew2_sb = consts.tile([96, MH], fp32, tag="me_w2")
        nc.sync.dma_start(out=mew2_sb[64:96, :], in_=ins["me_w2"])
        meb1c = consts.tile([96, 1], fp32, tag="me_b1c")
        nc.sync.dma_start(out=meb1c[64:96, :], in_=ins["me_b1c"])
        meb2c = consts.tile([96, 1], fp32, tag="me_b2c")
        nc.sync.dma_start(out=meb2c[64:96, :], in_=ins["me_b2c"])
        megc = consts.tile([96, 1], fp32, tag="me_gc")
        nc.sync.dma_start(out=megc[64:96, :], in_=ins["me_gc"])
        mebtc = consts.tile([96, 1], fp32, tag="me_btc")
        nc.sync.dma_start(out=mebtc[64:96, :], in_=ins["me_btc"])
        # broadcasts
        gng_b = consts.tile([128, GH], fp32, tag="gng_b")
        nc.gpsimd.dma_start(out=gng_b, in_=ins["gn_g_row"].to_broadcast((128, GH)))
        gnb_b = consts.tile([128, GH], fp32, tag="gnb_b")
        nc.gpsimd.dma_start(out=gnb_b, in_=ins["gn_b_row"].to_broadcast((128, GH)))
        b3_b = consts.tile([BL, V], fp32, tag="b3_b")
        nc.gpsimd.dma_start(out=b3_b, in_=ins["ph_b3_row"].to_broadcast((BL, V)))
        # small synthesized constants
        inv64 = consts.tile([GH, GH], fp32, tag="inv64")
        nc.vector.memset(inv64, 1.0 / GH)
        inv32 = consts.tile([96, MH], fp32, tag="inv32")
        nc.vector.memset(inv32[64:96, :], 1.0 / MH)
        eps64 = consts.tile([GH, 1], fp32, tag="eps64")
        nc.vector.memset(eps64, 1e-5)
        eps96 = consts.tile([96, 1], fp32, tag="eps96")
        nc.vector.memset(eps96, 1e-5)
        eps128 = consts.tile([128, 1], fp32, tag="eps128")
        nc.vector.memset(eps128, 1e-5)
        ones_r = consts.tile([97, 128], bf16, tag="ones_r")
        nc.vector.memset(ones_r, 1.0)
        ones_c = consts.tile([128, 1], fp32, tag="ones_c")
        nc.vector.memset(ones_c, 1.0)

        # ---------------- asset encoder ----------------
        xa_es = contextlib.ExitStack()
        xa_pool = xa_es.enter_context(tc.tile_pool(name="xa", bufs=1))
        enc_pool = xa_es.enter_context(tc.tile_pool(name="enc", bufs=1))
        xaT_sb = xa_pool.tile([KP, KC, R], bf16)
        for kc in range(KC):
            nc.sync.dma_start(
                out=xaT_sb[:, kc, :],
                in_=ins["xaT"][kc * KP:(kc + 1) * KP, :])
        h1T = enc_pool.tile([2 * GH, R], fp32)
        embpre = enc_pool.tile([GH, R], fp32)
        embT = persist.tile([GH, R], fp32)
        for pass_ in range(2):
            h1ps = []
            for pi in range(2):
                h1ps.append(ps_big.tile([128, 512], fp32, tag="zb", name="h1ps"))
            for kc in range(KC):
                for pi in range(2):
                    p = pass_ * 2 + pi
                    nc.tensor.matmul(h1ps[pi], w1_sb[:, kc, :],
                                     xaT_sb[:, kc, p * 512:(p + 1) * 512],
                                     start=(kc == 0), stop=(kc == KC - 1))
            for pi in range(2):
                p = pass_ * 2 + pi
                _act(nc, h1T[:, p * 512:(p + 1) * 512], h1ps[pi], gelu_fn, bias=b1c)
        for p in range(4):
            sl = slice(p * 512, (p + 1) * 512)
            m2ps = ps_out.tile([GH, 512], fp32, tag="outT")
            nc.tensor.matmul(m2ps, w2_sb, h1T[:, sl], start=True, stop=True)
            _act(nc, embpre[:, sl], m2ps, gelu_fn, bias=b2c)
            _ln_T(nc, pools, embpre[:, sl], embT[:, sl], GH, 512,
                  inv64, eps64, aeg, aebt)
        if dbg:
            nc.sync.dma_start(out=dbg["d_embT"], in_=embT)
        xa_es.close()

        # ---------------- macro encoder (partitions 64:96) ----------------
        mac_pool = tc.tile_pool(name="mac", bufs=1)
        with mac_pool as macp:
            xmT_sb = macp.tile([96, BL * L], fp32)
            nc.sync.dma_start(out=xmT_sb[64:96, :], in_=ins["xmT"])
            m1ps = ps_small.tile([96, BL * L], fp32, tag="small")
            nc.tensor.matmul(m1ps[64:96, :], mew1_sb[64:96, :], xmT_sb[64:96, :],
                             start=True, stop=True)
            m1s = macp.tile([96, BL * L], fp32)
            _act(nc, m1s[64:96, :], m1ps[64:96, :], gelu_fn, bias=meb1c[64:96, :])
            m2ps = ps_small.tile([96, BL * L], fp32, tag="small")
            nc.tensor.matmul(m2ps[64:96, :], mew2_sb[64:96, :], m1s[64:96, :],
                             start=True, stop=True)
            m2s = macp.tile([96, BL * L], fp32)
            _act(nc, m2s[64:96, :], m2ps[64:96, :], AF.Identity, bias=meb2c[64:96, :])
            macpre = macp.tile([96, BL], fp32)
            nc.vector.tensor_reduce(
                macpre[64:96, :],
                m2s[64:96, :].rearrange("p (b l) -> p b l", b=BL),
                axis=mybir.AxisListType.X, op=ALU.add)
            nc.vector.tensor_scalar(macpre[64:96, :], macpre[64:96, :],
                                    1.0 / L, None, ALU.mult)
            macT = persist.tile([96, BL], fp32)
            _ln_T(nc, pools, macpre[64:96, :], macT[64:96, :], MH, BL,
                  inv32[64:96, :], eps96[64:96, :], megc[64:96, :], mebtc[64:96, :],
                  pbase=64)
            if dbg:
                nc.sync.dma_start(out=dbg["d_mac"], in_=macT[64:96, :])

        # ---------------- GAT layers ----------------
        gat_es = contextlib.ExitStack()
        wxo_pool = gat_es.enter_context(tc.tile_pool(name="wxo", bufs=1))
        qt_pool = gat_es.enter_context(tc.tile_pool(name="qt", bufs=6))
        un_pool = gat_es.enter_context(tc.tile_pool(name="un", bufs=4))
        gp_pool = gat_es.enter_context(tc.tile_pool(name="gatp", bufs=1))

        hg1 = gp_pool.tile([128, NCH, GH], fp32, tag="hg1")
        h1gT = persist.tile([GH, R], fp32)
        h2acc = gp_pool.tile([128, NCH, GH], fp32, tag="h2acc")

        def gat_layer(layer, src_T, D):
            dp1 = D + 1
            gw = g1w_sb if layer == 1 else g2w_sb
            # --- prep: Wx in token-major layout (+ones col), q/u/z from st ---
            wxo = [wxo_pool.tile([128, NCH, dp1], bf16, tag=f"wxo{h}", name=f"wxo{h}")
                   for h in range(H)]
            for h in range(H):
                nc.vector.memset(wxo[h][:, :, D:dp1], 1.0)
            q_sb = gp_pool.tile([128, NCH, H], fp32, tag="q_sb")
            u_sb = gp_pool.tile([128, NCH, H], fp32, tag="u_sb")
            nu_sb = gp_pool.tile([128, NCH, H], fp32, tag="nu_sb")
            z_sb = gp_pool.tile([97, R], bf16, tag="z_sb")
            for rc in range(NCH):
                wxps = ps_out.tile([128, H * D], fp32, tag="outT")
                nc.tensor.matmul(wxps, src_T[:, rc * 128:(rc + 1) * 128], gw,
                                 start=True, stop=True)
                for h in range(H):
                    nc.any.tensor_copy(wxo[h][:, rc, 0:D],
                                       wxps[:, h * D:(h + 1) * D])
                stps = ps_tn.tile([128, 8], fp32, tag="tn")
                nc.tensor.matmul(stps, src_T[:, rc * 128:(rc + 1) * 128],
                                 wt_sb[layer], start=True, stop=True)
                _act(nc, q_sb[:, rc, :], stps[:, 4:8], AF.Exp)
                _act(nc, u_sb[:, rc, :], stps[:, 4:8], AF.Exp, scale=0.2)
            nc.vector.tensor_scalar(nu_sb, u_sb, -1.0, None, ALU.mult)
            u_bf = gp_pool.tile([128, NCH, H], bf16, tag="u_bf")
            nc.vector.tensor_copy(u_bf, u_sb)
            for b in range(BL):
                zps = ps_big.tile([97, 512], fp32, tag="zb")
                nc.tensor.matmul(zps, wts_sb[layer],
                                 src_T[:, b * 512:(b + 1) * 512],
                                 start=True, stop=True)
                for h in range(H):
                    _act(nc, z_sb[32 * h:32 * h + 1, b * 512:(b + 1) * 512],
                         zps[32 * h:32 * h + 1, :], AF.Exp, scale=0.8)
            # --- attention units ---
            for b in range(BL):
                for h in range(H):
                    u_idx = b * H + h
                    use_act = (u_idx % 16) < ACT_OF_16
                    zbps = ps_big.tile([128, 512], fp32, tag="zb")
                    nc.tensor.matmul(zbps, ones_r[32 * h:32 * h + 1, :],
                                     z_sb[32 * h:32 * h + 1, b * 512:(b + 1) * 512],
                                     start=True, stop=True,
                                     tile_position=(32 * h, 0))
                    outps = ps_out.tile([dp1, 512], fp32, tag="outT")
                    corr = None
                    if use_act:
                        corrps = ps_small.tile([dp1, 1], fp32, tag="small")
                    for jc in range(BL):
                        rc = b * 4 + jc
                        qt = qt_pool.tile([128, 512], bf16, tag="qt")
                        if use_act:
                            _act(nc, qt, zbps, AF.Relu,
                                 bias=nu_sb[:, rc, h:h + 1],
                                 scale=q_sb[:, rc, h:h + 1])
                        else:
                            nc.vector.tensor_scalar(
                                qt, zbps, q_sb[:, rc, h:h + 1],
                                u_sb[:, rc, h:h + 1], ALU.mult, ALU.max)
                        nc.tensor.matmul(outps, wxo[h][:, rc, :], qt,
                                         start=(jc == 0), stop=(jc == 3))
                        if use_act:
                            nc.tensor.matmul(corrps, wxo[h][:, rc, :],
                                             u_bf[:, rc, h:h + 1],
                                             start=(jc == 0), stop=(jc == 3))
                    outsb = un_pool.tile([dp1, 512], fp32, tag="outsb")
                    if use_act:
                        corr = un_pool.tile([dp1, 1], fp32, tag="corr")
                        nc.vector.tensor_copy(corr, corrps)
                        nc.vector.tensor_scalar(outsb, outps, corr, None, ALU.add)
                    else:
                        _act(nc, outsb, outps, AF.Copy)
                    for ic in range(BL):
                        icg = b * 4 + ic
                        tnps = ps_tn.tile([128, dp1], fp32, tag="tn")
                        nc.tensor.transpose(tnps, outsb[:, ic * 128:(ic + 1) * 128],
                                            ident[:dp1, :dp1])
                        rz = un_pool.tile([128, 1], fp32, tag="rz")
                        nc.vector.reciprocal(rz, tnps[:, D:dp1])
                        if layer == 1:
                            nc.vector.tensor_scalar(
                                hg1[:, icg, h * D:(h + 1) * D], tnps[:, 0:D],
                                rz, None, ALU.mult)
                        else:
                            if h == 0:
                                nc.vector.tensor_scalar(
                                    h2acc[:, icg, :], tnps[:, 0:D],
                                    rz, 0.25, ALU.mult, ALU.mult)
                            else:
                                tmp = un_pool.tile([128, D], fp32, tag="tmp")
                                nc.vector.tensor_scalar(
                                    tmp, tnps[:, 0:D], rz, 0.25, ALU.mult, ALU.mult)
                                nc.vector.tensor_tensor(
                                    h2acc[:, icg, :], h2acc[:, icg, :], tmp, ALU.add)

        def elu_inplace(x_sb):
            ex = gp_pool.tile([128, NCH, GH], fp32, tag="elu_ex")
            _act(nc, ex, x_sb, AF.Exp)
            nc.vector.tensor_scalar(ex, ex, 1.0, None, ALU.subtract)
            rel = gp_pool.tile([128, NCH, GH], fp32, tag="elu_rel")
            nc.vector.tensor_scalar(rel, x_sb, 0.0, None, ALU.max)
            nc.vector.tensor_tensor(x_sb, rel, ex, ALU.min)

        gat_layer(1, embT, D1)
        elu_inplace(hg1)
        for rc in range(NCH):
            tpps = ps_tn.tile([GH, 128], fp32, tag="tn")
            nc.tensor.transpose(tpps, hg1[:, rc, :], ident)
            nc.any.tensor_copy(h1gT[:, rc * 128:(rc + 1) * 128], tpps)
        if dbg:
            nc.sync.dma_start(out=dbg["d_h1gT"], in_=h1gT)

        gat_layer(2, h1gT, D2)
        elu_inplace(h2acc)

        # graph LayerNorm (free-dim LN, natural layout)
        ge = gp_pool.tile([128, NCH, GH], fp32, tag="ge")
        for rc in range(NCH):
            bst = scr.tile([128, 6], fp32, tag="bst")
            nc.vector.bn_stats(bst, h2acc[:, rc, :])
            mv = scr.tile([128, 2], fp32, tag="mv")
            nc.vector.bn_aggr(mv, bst)
            std = scr.tile([128, 1], fp32, tag="std")
            _act(nc, std, mv[:, 1:2], AF.Sqrt, bias=eps128)
            rstd = scr.tile([128, 1], fp32, tag="rstd")
            nc.vector.reciprocal(rstd, std)
            t1 = scr.tile([128, GH], fp32, tag="lngt1")
            nc.vector.tensor_scalar(t1, h2acc[:, rc, :], mv[:, 0:1], rstd,
                                    ALU.subtract, ALU.mult)
            nc.vector.tensor_tensor(t1, t1, gng_b, ALU.mult)
            nc.vector.tensor_tensor(ge[:, rc, :], t1, gnb_b, ALU.add)
            if dbg:
                nc.sync.dma_start(out=dbg["d_ge"][rc, :, :], in_=ge[:, rc, :])
        gat_es.close()

        # ---------------- pool + head ----------------
        with tc.tile_pool(name="head", bufs=1) as hp:
            poolps = ps_out.tile([GH, BL], fp32, tag="outT")
            for b in range(BL):
                for rc4 in range(4):
                    nc.tensor.matmul(poolps[:, b:b + 1], ge[:, b * 4 + rc4, :],
                                     ones_c, start=(rc4 == 0), stop=(rc4 == 3))
            comb = hp.tile([GH + MH, BL], fp32)
            nc.vector.tensor_scalar(comb[0:GH, :], poolps, 1.0 / NA, None, ALU.mult)
            nc.vector.tensor_copy(comb[GH:GH + MH, :], macT[64:96, :])
            z1ps = ps_big.tile([PH, BL], fp32, tag="zb")
            nc.tensor.matmul(z1ps, phw1_sb, comb, start=True, stop=True)
            z1s = hp.tile([PH, BL], fp32)
            _act(nc, z1s, z1ps, gelu_fn, bias=phb1c)
            z2ps = ps_tn.tile([PH // 2, BL], fp32, tag="tn")
            nc.tensor.matmul(z2ps, phw2_sb, z1s, start=True, stop=True)
            z2s = hp.tile([PH // 2, BL], fp32)
            _act(nc, z2s, z2ps, gelu_fn, bias=phb2c)
            lgps = ps_out.tile([BL, V], fp32, tag="outT")
            nc.tensor.matmul(lgps, z2s, phw3_sb, start=True, stop=True)
            sm = hp.tile([BL, V], fp32)
            nc.vector.tensor_add(sm, lgps, b3_b)
            mx = hp.tile([BL, 1], fp32)
            nc.vector.tensor_reduce(mx, sm, axis=mybir.AxisListType.X, op=ALU.max)
            nc.vector.tensor_scalar(mx, mx, -1.0, None, ALU.mult)
            ex = hp.tile([BL, V], fp32)
            _act(nc, ex, sm, AF.Exp, bias=mx)
            sme = hp.tile([BL, 1], fp32)
            nc.vector.tensor_reduce(sme, ex, axis=mybir.AxisListType.X, op=ALU.add)
            rs = hp.tile([BL, 1], fp32)
            nc.vector.reciprocal(rs, sme)
            res = hp.tile([BL, V], fp32)
            nc.vector.tensor_scalar(res, ex, rs, None, ALU.mult)
            nc.sync.dma_start(out=out_d, in_=res)


# ---------------- host side ----------------

def host_prep(inputs):
    """Build the per-core in_maps from full (unsharded) numpy inputs."""
    f32 = np.float32
    x_asset = np.ascontiguousarray(inputs["x_asset"], dtype=f32)
    x_macro = np.ascontiguousarray(inputs["x_macro"], dtype=f32)
    g1_w, g1_a = np.asarray(inputs["g1_w"], f32), np.asarray(inputs["g1_a"], f32)
    g2_w, g2_a = np.asarray(inputs["g2_w"], f32), np.asarray(inputs["g2_a"], f32)

    def fold(gw, ga, D):
        ws = np.stack([gw[:, h * D:(h + 1) * D] @ ga[h, :D] for h in range(H)], 1)
        wtt = np.stack([gw[:, h * D:(h + 1) * D] @ ga[h, D:] for h in range(H)], 1)
        wt = np.concatenate([ws, wtt], axis=1).astype(f32)          # (GH, 8)
        wspread = np.zeros((GH, 97), f32)
        for h in range(H):
            wspread[:, 32 * h] = ws[:, h]
        return np.ascontiguousarray(wt), np.ascontiguousarray(wspread)

    wt1, wts1 = fold(g1_w, g1_a, D1)
    wt2, wts2 = fold(g2_w, g2_a, D2)

    col = lambda v: np.ascontiguousarray(np.asarray(v, f32).reshape(-1, 1))
    row = lambda v: np.ascontiguousarray(np.asarray(v, f32).reshape(1, -1))
    import ml_dtypes
    bf = ml_dtypes.bfloat16
    shared = {
        "ae_w1": np.ascontiguousarray(np.asarray(inputs["ae_w1"], f32).astype(bf)),
        "ae_b1c": col(inputs["ae_b1"]), "ae_w2": np.ascontiguousarray(inputs["ae_w2"], f32),
        "ae_b2c": col(inputs["ae_b2"]), "ae_gc": col(inputs["ae_g"]),
        "ae_btc": col(inputs["ae_bt"]),
        "me_w1": np.ascontiguousarray(inputs["me_w1"], f32), "me_b1c": col(inputs["me_b1"]),
        "me_w2": np.ascontiguousarray(inputs["me_w2"], f32), "me_b2c": col(inputs["me_b2"]),
        "me_gc": col(inputs["me_g"]), "me_btc": col(inputs["me_bt"]),
        "g1_w": g1_w, "g2_w": g2_w, "wt1": wt1, "wt2": wt2,
        "wts1": wts1, "wts2": wts2,
        "gn_g_row": row(inputs["gn_g"]), "gn_b_row": row(inputs["gn_b"]),
        "ph_w1": np.ascontiguousarray(inputs["ph_w1"], f32), "ph_b1c": col(inputs["ph_b1"]),
        "ph_w2": np.ascontiguousarray(inputs["ph_w2"], f32), "ph_b2c": col(inputs["ph_b2"]),
        "ph_w3": np.ascontiguousarray(inputs["ph_w3"], f32), "ph_b3_row": row(inputs["ph_b3"]),
        "ident": np.eye(128, dtype=f32),
    }
    in_maps = []
    for c in range(N_CORES):
        xa = x_asset[c * BL:(c + 1) * BL].reshape(R, KD)
        xm = x_macro[c * BL:(c + 1) * BL].reshape(BL * L, NM)
        m = dict(shared)
        m["xaT"] = np.ascontiguousarray(xa.T.astype(bf))
        m["xmT"] = np.ascontiguousarray(xm.T)
        in_maps.append(m)
    return in_maps


_NC_CACHE = {}


def _get_program(debug=False):
    if debug not in _NC_CACHE:
        _NC_CACHE[debug] = build_program(debug=debug)
    return _NC_CACHE[debug]


def kernel(**inputs) -> np.ndarray:
    nc = _get_program(debug=False)
    in_maps = host_prep(inputs)
    res = bass_utils.run_bass_kernel_spmd(nc, in_maps, core_ids=list(range(N_CORES)))
    return np.concatenate([res.results[c]["out"] for c in range(N_CORES)], axis=0)
